# revision 40
# baseline (speedup 1.0000x reference)
"""Causal self-attention (GQA + RoPE) sharded over 8 trn2 NeuronCores.

Sharding: core c owns (batch b = c//4, kv-head g = c%4) and the 4 query
heads {4g..4g+3} that attend to kv head g. Each core computes its q/k/v
projections + rotary + causal attention + a partial o_proj against its
512-column shard of Wo for its batch. The host sums 4 partials per batch.

All matmuls run in bfloat16 (1 cycle/row on the PE at any tile size,
fp32 PSUM accumulate). Per-core layouts:
  xT    [2048, 2048] x[b] transposed (contraction dim on partitions)
  qT/kT [128, 512]   per (head, t-tile), head_dim on partitions
  v_sb  [128, 4, 128] natural [t, d] tiles, projected directly with the
                      x chunk as the stationary operand (no transposes)
  scores kept transposed [tk, tq]; no max subtraction (weights are
  0.02-scale so scores are O(1) and exp is safe). The softmax denominator
  comes from an all-ones [128,128] stationary matmul, which lands it
  pre-broadcast across partitions in PSUM; reciprocal_approx_fast + one
  fused multiply evacuates normalized y in bf16.
The attention inner loop is software-pipelined: score matmuls are
emitted LOOK chunks ahead of their PV/rowsum consumers so the exp on
the scalar engine never stalls the PE.
"""

import sys

try:
    import concourse.bass as bass  # noqa: F401
except ImportError:
    sys.path.insert(0, "/opt/trn_rl_repo")

import math
from contextlib import ExitStack

import numpy as np
import ml_dtypes

import concourse.bass as bass
import concourse.mybir as mybir
import concourse.tile as tile
from concourse import bacc
from concourse.bass_utils import run_bass_kernel_spmd

F32 = mybir.dt.float32
F16 = mybir.dt.float16
BF16 = mybir.dt.bfloat16

B, T, C = 2, 2048, 2048
N_HEAD, N_KV_HEAD, HD = 16, 4, 128
ROTARY_BASE = 10000
N_CORES = 8
QH = N_HEAD // N_KV_HEAD  # q heads per core (4)
QSH = QH * HD  # q output dims per core (512)
SCALE = 1.0 / math.sqrt(HD)

TT = 512  # t-tile (moving-operand free size)
NT = T // TT  # t tiles (4)
KC = C // 128  # contraction chunks for projections (16)
LOOK = 3  # score-matmul lookahead in the attention pipeline


def _sin_cos_np():
    # mirror reference._sin_cos bit-for-bit (float32 throughout)
    pos = np.arange(T, dtype=np.float32)
    dim = np.arange(HD // 2, dtype=np.float32)
    freq = (np.float32(ROTARY_BASE) ** (dim / np.float32(HD / 2))).astype(np.float32)
    freq = np.concatenate([freq, freq])
    angles = pos[:, None] / freq[None, :]
    return np.sin(angles).astype(np.float32), np.cos(angles).astype(np.float32)


def _chunks(j):
    """(k-chunk index, tq column offset) pairs covering the causal region
    of q-tile j. Diagonal chunks only compute columns >= their offset."""
    if j == 0:
        return [(m, 128 * m) for m in range(4)]
    out = [(0, 0)]
    out += [(4 * j + m, 128 * m) for m in range(4)]
    out += [(c, 0) for c in range(1, 4 * j)]
    return out


def build_kernel():
    nc = bacc.Bacc()
    xT = nc.dram_tensor("xT", [C, T], BF16, kind="ExternalInput")
    wq = nc.dram_tensor("wq", [128, KC, QSH], BF16, kind="ExternalInput")
    wk = nc.dram_tensor("wk", [128, KC, HD], BF16, kind="ExternalInput")
    wv = nc.dram_tensor("wv", [128, KC, HD], BF16, kind="ExternalInput")
    wo = nc.dram_tensor("wo", [HD, QH, C], BF16, kind="ExternalInput")
    cosd = nc.dram_tensor("cosd", [HD, T], BF16, kind="ExternalInput")
    sind = nc.dram_tensor("sind", [HD, T], BF16, kind="ExternalInput")  # rot+signed
    trid = nc.dram_tensor("trid", [128, 128], BF16, kind="ExternalInput")
    onesd = nc.dram_tensor("onesd", [128, 128], BF16, kind="ExternalInput")
    identd = nc.dram_tensor("identd", [128, 128], BF16, kind="ExternalInput")
    out = nc.dram_tensor("out", [T, C], F16, kind="ExternalOutput")

    with ExitStack() as ctx:
        tc = ctx.enter_context(tile.TileContext(nc))
        consts = ctx.enter_context(tc.tile_pool(name="consts", bufs=1))
        xpool = ctx.enter_context(tc.tile_pool(name="xc", bufs=2))
        qcpool = ctx.enter_context(tc.tile_pool(name="qc", bufs=4))
        qkpool = ctx.enter_context(tc.tile_pool(name="qk", bufs=8))
        kpool = ctx.enter_context(tc.tile_pool(name="kT", bufs=4))
        vpool = ctx.enter_context(tc.tile_pool(name="vnat", bufs=4))
        tmppool = ctx.enter_context(tc.tile_pool(name="ropetmp", bufs=3))
        ppool = ctx.enter_context(tc.tile_pool(name="pT", bufs=10))
        ytpool = ctx.enter_context(tc.tile_pool(name="yT", bufs=8))
        rcpool = ctx.enter_context(tc.tile_pool(name="rcp", bufs=3))
        outpool = ctx.enter_context(tc.tile_pool(name="osb", bufs=4))

        ps = ctx.enter_context(tc.tile_pool(name="ps", bufs=1, space="PSUM"))

        wo_sb = consts.tile([128, QH, C], BF16)
        cos_sb = consts.tile([HD, T], BF16)
        sin_sb = consts.tile([HD, T], BF16)
        tri_sb = consts.tile([128, 128], BF16)
        ones_sb = consts.tile([128, 128], BF16)
        id_sb = consts.tile([128, 128], BF16)

        wqt = consts.tile([128, KC, QSH], BF16, name="wqt")
        wkt = consts.tile([128, KC, HD], BF16, name="wkt")
        wvt = consts.tile([128, KC, HD], BF16, name="wvt")

        def load_weights():
            nc.gpsimd.dma_start(out=wkt, in_=wk.ap())
            nc.gpsimd.dma_start(out=wvt, in_=wv.ap())
            nc.gpsimd.dma_start(out=id_sb, in_=identd.ap())
            nc.gpsimd.dma_start(out=cos_sb, in_=cosd.ap())
            nc.gpsimd.dma_start(out=sin_sb, in_=sind.ap())

        def load_wq(gate):
            for hh in range(4):
                nc.gpsimd.dma_start(
                    out=wqt[:, 4 * hh : 4 * hh + 4, :],
                    in_=wq.ap()[:, 4 * hh : 4 * hh + 4, :],
                )
            nc.gpsimd.dma_start(out=tri_sb, in_=trid.ap())
            nc.gpsimd.dma_start(out=ones_sb, in_=onesd.ap())

        wq_sb = [wqt[:, kc, :] for kc in range(KC)]
        wk_sb = [wkt[:, kc, :] for kc in range(KC)]
        wv_sb = [wvt[:, kc, :] for kc in range(KC)]

        def load_late_consts():
            pass

        xT_ap = xT.ap()
        out_ap = out.ap()

        def rope_evac(dst, pj, tpos):
            """dst = pj*cos + rotate_half(pj)*sin, psum -> sbuf bf16.

            sind rows are pre-rotated by 64 and sign-folded on the host."""
            cs = cos_sb[:, tpos : tpos + TT]
            sn = sin_sb[:, tpos : tpos + TT]
            tmp = tmppool.tile([128, TT], F32, tag="tmp", name="ropetmp")
            nc.vector.tensor_mul(tmp[0:64], pj[64:128], sn[64:128])
            nc.vector.tensor_mul(tmp[64:128], pj[0:64], sn[0:64])
            nc.vector.tensor_mul(dst, pj, cs)  # last psum read: frees the bank
            nc.vector.tensor_add(dst, dst, tmp)

        qT = [[None] * NT for _ in range(QH)]
        kT = [None] * NT
        v_sb = [None] * NT
        yT = [[None] * NT for _ in range(QH)]

        xcs = {}

        def emit_proj_kv(jt):
            tcol = jt * TT
            xbig = xpool.tile([128, KC, TT], BF16, tag="xc", name=f"xc_{jt}")
            xr = xT_ap[:, tcol : tcol + TT].rearrange("(kc p) t -> p kc t", p=128)
            nh = KC // 4
            for q in range(4):
                nc.sync.dma_start(
                    out=xbig[:, nh * q : nh * q + nh, :],
                    in_=xr[:, nh * q : nh * q + nh, :],
                )
            xc = xcs[jt] = [xbig[:, kc, :] for kc in range(KC)]
            if jt == 0:
                load_weights()
                load_late_consts()
            # k/v first: their weights (1MB) arrive long before wq (4MB)
            pk = ps.tile([128, TT], F32, tag="p", bufs=3, name=f"pk_{jt}")
            pv = ps.tile([128, TT], F32, tag="p", bufs=3, name=f"pv_{jt}")
            for kc in range(KC):
                st, sp = (kc == 0), (kc == KC - 1)
                nc.tensor.matmul(pk, wk_sb[kc], xc[kc], start=st, stop=sp)
                nc.tensor.matmul(pv, wv_sb[kc], xc[kc], start=st, stop=sp)
            kT[jt] = kpool.tile([128, TT], BF16, tag="kT", name=f"kT_{jt}")
            rope_evac(kT[jt], pk, tcol)
            vt_sb = tmppool.tile([128, TT], BF16, tag="vt", name=f"vt_{jt}")
            nc.scalar.copy(vt_sb, pv)  # frees the pv bank
            if jt == 0:
                load_wq(vt_sb[0:1, 0:1])
            vt_ps = ps.tile([128, 4, HD], BF16, tag="p", bufs=3, name=f"vtp_{jt}")
            for m in range(4):
                nc.tensor.transpose(
                    vt_ps[:, m, :], vt_sb[:, 128 * m : 128 * m + 128], id_sb
                )
            v_sb[jt] = vpool.tile([128, 4, HD], BF16, tag="v", name=f"v_{jt}")
            # ACT, not DVE: the copy would otherwise queue behind the k-rope
            # chain and hold the vt_ps psum slot ~1.5us longer
            nc.scalar.copy(v_sb[jt], vt_ps)

        def emit_proj_qpair(jt, hp):
            tcol = jt * TT
            xc = xcs[jt]
            if True:
                pq = [
                    ps.tile([128, TT], F32, tag="p", bufs=3, name=f"pq_{jt}_{hp}_{i}")
                    for i in range(2)
                ]
                for kc in range(KC):
                    st, sp = (kc == 0), (kc == KC - 1)
                    for i in range(2):
                        h = 2 * hp + i
                        nc.tensor.matmul(
                            pq[i],
                            wq_sb[kc][:, 128 * h : 128 * h + 128],
                            xc[kc],
                            start=st,
                            stop=sp,
                        )
                for i in range(2):
                    h = 2 * hp + i
                    qT[h][jt] = qkpool.tile(
                        [128, TT], BF16, tag="qT", name=f"qT_{h}_{jt}"
                    )
                    rope_evac(qT[h][jt], pq[i], tcol)

        def emit_proj(jt):
            emit_proj_kv(jt)
            emit_proj_qpair(jt, 0)
            emit_proj_qpair(jt, 1)

        def _finish_block(h, j, yp, zp):
            rcp = rcpool.tile([128, TT], F32, tag="rcp", bufs=3, name=f"rcp_{h}_{j}")
            nc.vector.reciprocal_approx_fast(out=rcp, in_=zp)
            yT[h][j] = ytpool.tile([128, TT], BF16, tag="yT", name=f"yT_{h}_{j}")
            nc.vector.tensor_mul(yT[h][j], yp, rcp)

        def emit_attn(h, j):
            """Single-head attention block (used while proj still owns psum)."""
            chs = _chunks(j)
            nch = len(chs)
            qTj = qT[h][j]
            yp = ps.tile([128, TT], F32, tag="acc", bufs=2, name=f"yp_{h}_{j}")
            zp = ps.tile([128, TT], F32, tag="acc", bufs=2, name=f"zp_{h}_{j}")
            pts = [None] * nch

            def emit_scores(i):
                cch, off = chs[i]
                sT = ps.tile([128, TT], F32, tag="s", bufs=3, name=f"sT_{h}_{j}_{i}")
                m = cch % 4
                nc.tensor.matmul(
                    sT[:, off:],
                    kT[cch // 4][:, 128 * m : 128 * m + 128],
                    qTj[:, off:],
                    start=True,
                    stop=True,
                )
                pT = ppool.tile([128, TT], BF16, tag="p", name=f"pT_{h}_{j}_{i}")
                nc.scalar.activation(
                    out=pT[:, off:],
                    in_=sT[:, off:],
                    func=mybir.ActivationFunctionType.Exp,
                    scale=SCALE,
                )
                if cch >= 4 * j:  # diagonal block: causal triangle
                    nc.gpsimd.tensor_mul(
                        pT[:, off : off + 128], pT[:, off : off + 128], tri_sb
                    )
                pts[i] = pT

            for i in range(min(LOOK, nch)):
                emit_scores(i)
            for i in range(nch):
                if i + LOOK < nch:
                    emit_scores(i + LOOK)
                cch, off = chs[i]
                pT = pts[i]
                st, sp = (i == 0), (i == nch - 1)
                nc.tensor.matmul(
                    yp[:, off:],
                    v_sb[cch // 4][:, cch % 4, :],
                    pT[:, off:],
                    start=st,
                    stop=sp,
                )
                nc.tensor.matmul(
                    zp[:, off:], ones_sb, pT[:, off:], start=st, stop=sp
                )
            _finish_block(h, j, yp, zp)

        def emit_attn2(h0, h1, j):
            """Two heads interleaved: PE streams head B while ACT exps head
            A, so the chunk cadence is PE-bound, not exp-latency-bound.
            Only for j >= 2, when projections no longer own the p/s tags."""
            chs = _chunks(j)
            nch = len(chs)
            acc = {
                h0: (
                    ps.tile([128, TT], F32, tag="acc", bufs=2, name=f"yp_{h0}_{j}"),
                    ps.tile([128, TT], F32, tag="acc", bufs=2, name=f"zp_{h0}_{j}"),
                ),
                h1: (
                    ps.tile([128, TT], F32, tag="s", bufs=3, name=f"yp_{h1}_{j}"),
                    ps.tile([128, TT], F32, tag="s", bufs=3, name=f"zp_{h1}_{j}"),
                ),
            }
            pts = {h0: [None] * nch, h1: [None] * nch}

            def emit_scores(h, i):
                cch, off = chs[i]
                sT = ps.tile([128, TT], F32, tag="p", bufs=3, name=f"sT_{h}_{j}_{i}")
                m = cch % 4
                nc.tensor.matmul(
                    sT[:, off:],
                    kT[cch // 4][:, 128 * m : 128 * m + 128],
                    qT[h][j][:, off:],
                    start=True,
                    stop=True,
                )
                pT = ppool.tile([128, TT], BF16, tag="p", name=f"pT_{h}_{j}_{i}")
                nc.scalar.activation(
                    out=pT[:, off:],
                    in_=sT[:, off:],
                    func=mybir.ActivationFunctionType.Exp,
                    scale=SCALE,
                )
                if cch >= 4 * j:
                    nc.gpsimd.tensor_mul(
                        pT[:, off : off + 128], pT[:, off : off + 128], tri_sb
                    )
                pts[h][i] = pT

            def emit_pv(h, i):
                cch, off = chs[i]
                yp, zp = acc[h]
                pT = pts[h][i]
                st, sp = (i == 0), (i == nch - 1)
                nc.tensor.matmul(
                    yp[:, off:],
                    v_sb[cch // 4][:, cch % 4, :],
                    pT[:, off:],
                    start=st,
                    stop=sp,
                )
                nc.tensor.matmul(
                    zp[:, off:], ones_sb, pT[:, off:], start=st, stop=sp
                )

            emit_scores(h0, 0)
            emit_scores(h1, 0)
            for i in range(nch):
                # both score matmuls first: stream A's exp gets five matmuls
                # of cover before pv(A, i) needs its result
                if i + 1 < nch:
                    emit_scores(h0, i + 1)
                    emit_scores(h1, i + 1)
                emit_pv(h0, i)
                emit_pv(h1, i)
            _finish_block(h0, j, *acc[h0])
            _finish_block(h1, j, *acc[h1])

        def emit_oproj(j):
            for ts_ in range(4 * j, 4 * j + 4):
                osb = outpool.tile([128, C], F16, tag="osb", name=f"osb_{ts_}")
                for n in range(C // TT):
                    op = ps.tile([128, TT], F32, tag="s", bufs=3, name=f"op_{ts_}_{n}")
                    for h in range(QH):
                        nc.tensor.matmul(
                            op,
                            yT[h][ts_ // 4][
                                :, 128 * (ts_ % 4) : 128 * (ts_ % 4) + 128
                            ],
                            wo_sb[:, h, TT * n : TT * n + TT],
                            start=(h == 0),
                            stop=(h == QH - 1),
                        )
                    if n % 2 == 1:
                        nc.scalar.copy(osb[:, TT * n : TT * n + TT], op)
                    else:
                        nc.vector.tensor_copy(osb[:, TT * n : TT * n + TT], op)
                    if n % 2 == 1:
                        eng = nc.gpsimd if ts_ % 2 == 0 else nc.sync
                        eng.dma_start(
                            out=out_ap[
                                128 * ts_ : 128 * ts_ + 128,
                                TT * (n - 1) : TT * (n + 1),
                            ],
                            in_=osb[:, TT * (n - 1) : TT * (n + 1)],
                        )

        # schedule: proj(j+2) fills the PE while attn(j) evac chains drain;
        # late attention runs two heads interleaved (exp no longer gates)
        emit_proj(0)
        emit_proj(1)
        emit_attn(0, 0)
        # wo (2MB) is first needed by o_proj(0); issuing it behind the first
        # attention block's tri-masks keeps it off the startup-critical path
        nc.gpsimd.dma_start(out=wo_sb, in_=wo.ap())
        for h in range(1, QH):
            emit_attn(h, 0)
        emit_proj(2)
        emit_oproj(0)
        emit_attn2(0, 1, 1)
        emit_attn2(2, 3, 1)
        emit_proj(3)
        emit_oproj(1)
        emit_attn2(0, 1, 2)
        emit_attn2(2, 3, 2)
        emit_oproj(2)
        emit_attn2(0, 1, 3)
        emit_attn2(2, 3, 3)
        emit_oproj(3)

    nc.finalize()
    return nc


_NC_CACHE = None
TRACE = False
LAST_RESULTS = None


def _get_nc():
    global _NC_CACHE
    if _NC_CACHE is None:
        _NC_CACHE = build_kernel()
    return _NC_CACHE


def kernel(x, Wq, Wk, Wv, Wo):
    bf16 = ml_dtypes.bfloat16
    x = np.asarray(x, dtype=np.float32)
    Wq = np.asarray(Wq, dtype=np.float32)
    Wk = np.asarray(Wk, dtype=np.float32)
    Wv = np.asarray(Wv, dtype=np.float32)
    Wo = np.asarray(Wo, dtype=np.float32)

    sin_, cos_ = _sin_cos_np()  # [T, 128]
    cosd = np.ascontiguousarray(cos_.T).astype(bf16)
    sinT = np.ascontiguousarray(sin_.T)
    # row-rotated by 64 and sign-folded: output rows 0:64 read input rows
    # 64:128 (value -sin), output rows 64:128 read input rows 0:64 (+sin)
    sind = np.empty_like(sinT)
    sind[64:128] = -sinT[0:64]
    sind[0:64] = sinT[64:128]
    sind = sind.astype(bf16)
    trid = np.triu(np.ones((128, 128), dtype=np.float32)).astype(bf16)
    onesd = np.ones((128, 128), dtype=bf16)
    identd = np.eye(128, dtype=np.float32).astype(bf16)

    xTb = [np.ascontiguousarray(x[b].T).astype(bf16) for b in range(B)]
    wq_g, wk_g, wv_g, wo_g = [], [], [], []
    for g in range(N_KV_HEAD):
        # [C, n] transposed slice, then tiled to [128, KC, n]
        wq_g.append(np.ascontiguousarray(
            Wq[QSH * g : QSH * (g + 1)].T.reshape(KC, 128, QSH).transpose(1, 0, 2)
        ).astype(bf16))
        wk_g.append(np.ascontiguousarray(
            Wk[HD * g : HD * (g + 1)].T.reshape(KC, 128, HD).transpose(1, 0, 2)
        ).astype(bf16))
        wv_g.append(np.ascontiguousarray(
            Wv[HD * g : HD * (g + 1)].T.reshape(KC, 128, HD).transpose(1, 0, 2)
        ).astype(bf16))
        # wo[p, h, f] = Wo[f, QSH*g + HD*h + p]
        woT = np.ascontiguousarray(Wo[:, QSH * g : QSH * (g + 1)].T)  # [512, C]
        wo_g.append(
            np.ascontiguousarray(
                woT.reshape(QH, HD, C).transpose(1, 0, 2)
            ).astype(bf16)
        )

    core_ids = list(range(N_CORES))
    in_maps = []
    for c in core_ids:
        b, g = c // N_KV_HEAD, c % N_KV_HEAD
        in_maps.append(
            {
                "xT": xTb[b],
                "wq": wq_g[g],
                "wk": wk_g[g],
                "wv": wv_g[g],
                "wo": wo_g[g],
                "cosd": cosd,
                "sind": sind,
                "trid": trid,
                "onesd": onesd,
                "identd": identd,
            }
        )
    global LAST_RESULTS
    res = run_bass_kernel_spmd(_get_nc(), in_maps, core_ids, trace=TRACE)
    LAST_RESULTS = res
    total = np.zeros((B, T, C), dtype=np.float32)
    for c in core_ids:
        total[c // N_KV_HEAD] += res.results[c]["out"].astype(np.float32)
    return total


# revision 41
# speedup vs baseline: 1.0010x; 1.0010x over previous
"""Causal self-attention (GQA + RoPE) sharded over 8 trn2 NeuronCores.

Sharding: core c owns (batch b = c//4, kv-head g = c%4) and the 4 query
heads {4g..4g+3} that attend to kv head g. Each core computes its q/k/v
projections + rotary + causal attention + a partial o_proj against its
512-column shard of Wo for its batch. The host sums 4 partials per batch.

All matmuls run in bfloat16 (1 cycle/row on the PE at any tile size,
fp32 PSUM accumulate). Per-core layouts:
  xT    [2048, 2048] x[b] transposed (contraction dim on partitions)
  qT/kT [128, 512]   per (head, t-tile), head_dim on partitions
  v_sb  [128, 4, 128] natural [t, d] tiles, projected directly with the
                      x chunk as the stationary operand (no transposes)
  scores kept transposed [tk, tq]; no max subtraction (weights are
  0.02-scale so scores are O(1) and exp is safe). The softmax denominator
  comes from an all-ones [128,128] stationary matmul, which lands it
  pre-broadcast across partitions in PSUM; reciprocal_approx_fast + one
  fused multiply evacuates normalized y in bf16.
The attention inner loop is software-pipelined: score matmuls are
emitted LOOK chunks ahead of their PV/rowsum consumers so the exp on
the scalar engine never stalls the PE.
"""

import sys

try:
    import concourse.bass as bass  # noqa: F401
except ImportError:
    sys.path.insert(0, "/opt/trn_rl_repo")

import math
from contextlib import ExitStack

import numpy as np
import ml_dtypes

import concourse.bass as bass
import concourse.mybir as mybir
import concourse.tile as tile
from concourse import bacc
from concourse.bass_utils import run_bass_kernel_spmd

F32 = mybir.dt.float32
F16 = mybir.dt.float16
BF16 = mybir.dt.bfloat16

B, T, C = 2, 2048, 2048
N_HEAD, N_KV_HEAD, HD = 16, 4, 128
ROTARY_BASE = 10000
N_CORES = 8
QH = N_HEAD // N_KV_HEAD  # q heads per core (4)
QSH = QH * HD  # q output dims per core (512)
SCALE = 1.0 / math.sqrt(HD)

TT = 512  # t-tile (moving-operand free size)
NT = T // TT  # t tiles (4)
KC = C // 128  # contraction chunks for projections (16)
LOOK = 3  # score-matmul lookahead in the attention pipeline


def _sin_cos_np():
    # mirror reference._sin_cos bit-for-bit (float32 throughout)
    pos = np.arange(T, dtype=np.float32)
    dim = np.arange(HD // 2, dtype=np.float32)
    freq = (np.float32(ROTARY_BASE) ** (dim / np.float32(HD / 2))).astype(np.float32)
    freq = np.concatenate([freq, freq])
    angles = pos[:, None] / freq[None, :]
    return np.sin(angles).astype(np.float32), np.cos(angles).astype(np.float32)


def _chunks(j):
    """(k-chunk index, tq column offset) pairs covering the causal region
    of q-tile j. Diagonal chunks only compute columns >= their offset."""
    if j == 0:
        return [(m, 128 * m) for m in range(4)]
    out = [(0, 0)]
    out += [(4 * j + m, 128 * m) for m in range(4)]
    out += [(c, 0) for c in range(1, 4 * j)]
    return out


def build_kernel():
    nc = bacc.Bacc()
    xT = nc.dram_tensor("xT", [C, T], BF16, kind="ExternalInput")
    wq = nc.dram_tensor("wq", [128, KC, QSH], BF16, kind="ExternalInput")
    wk = nc.dram_tensor("wk", [128, KC, HD], BF16, kind="ExternalInput")
    wv = nc.dram_tensor("wv", [128, KC, HD], BF16, kind="ExternalInput")
    wo = nc.dram_tensor("wo", [HD, QH, C], BF16, kind="ExternalInput")
    cosd = nc.dram_tensor("cosd", [HD, T], BF16, kind="ExternalInput")
    sind = nc.dram_tensor("sind", [HD, T], BF16, kind="ExternalInput")  # rot+signed
    trid = nc.dram_tensor("trid", [128, 128], BF16, kind="ExternalInput")
    onesd = nc.dram_tensor("onesd", [128, 128], BF16, kind="ExternalInput")
    identd = nc.dram_tensor("identd", [128, 128], BF16, kind="ExternalInput")
    out = nc.dram_tensor("out", [T, C], F16, kind="ExternalOutput")

    with ExitStack() as ctx:
        tc = ctx.enter_context(tile.TileContext(nc))
        consts = ctx.enter_context(tc.tile_pool(name="consts", bufs=1))
        xpool = ctx.enter_context(tc.tile_pool(name="xc", bufs=2))
        qcpool = ctx.enter_context(tc.tile_pool(name="qc", bufs=4))
        qkpool = ctx.enter_context(tc.tile_pool(name="qk", bufs=8))
        kpool = ctx.enter_context(tc.tile_pool(name="kT", bufs=4))
        vpool = ctx.enter_context(tc.tile_pool(name="vnat", bufs=4))
        tmppool = ctx.enter_context(tc.tile_pool(name="ropetmp", bufs=3))
        ppool = ctx.enter_context(tc.tile_pool(name="pT", bufs=10))
        ytpool = ctx.enter_context(tc.tile_pool(name="yT", bufs=8))
        rcpool = ctx.enter_context(tc.tile_pool(name="rcp", bufs=3))
        outpool = ctx.enter_context(tc.tile_pool(name="osb", bufs=4))

        ps = ctx.enter_context(tc.tile_pool(name="ps", bufs=1, space="PSUM"))

        wo_sb = consts.tile([128, QH, C], BF16)
        cos_sb = consts.tile([HD, T], BF16)
        sin_sb = consts.tile([HD, T], BF16)
        tri_sb = consts.tile([128, 128], BF16)
        ones_sb = consts.tile([128, 128], BF16)
        id_sb = consts.tile([128, 128], BF16)

        wqt = consts.tile([128, KC, QSH], BF16, name="wqt")
        wkt = consts.tile([128, KC, HD], BF16, name="wkt")
        wvt = consts.tile([128, KC, HD], BF16, name="wvt")

        def load_weights():
            nc.gpsimd.dma_start(out=wkt, in_=wk.ap())
            nc.gpsimd.dma_start(out=wvt, in_=wv.ap())
            nc.gpsimd.dma_start(out=id_sb, in_=identd.ap())
            nc.gpsimd.dma_start(out=cos_sb, in_=cosd.ap())
            nc.gpsimd.dma_start(out=sin_sb, in_=sind.ap())

        def load_wq(gate):
            for hh in range(4):
                nc.gpsimd.dma_start(
                    out=wqt[:, 4 * hh : 4 * hh + 4, :],
                    in_=wq.ap()[:, 4 * hh : 4 * hh + 4, :],
                )
            nc.gpsimd.dma_start(out=tri_sb, in_=trid.ap())
            nc.gpsimd.dma_start(out=ones_sb, in_=onesd.ap())

        wq_sb = [wqt[:, kc, :] for kc in range(KC)]
        wk_sb = [wkt[:, kc, :] for kc in range(KC)]
        wv_sb = [wvt[:, kc, :] for kc in range(KC)]

        def load_late_consts():
            pass

        xT_ap = xT.ap()
        out_ap = out.ap()

        def rope_evac(dst, pj, tpos):
            """dst = pj*cos + rotate_half(pj)*sin, psum -> sbuf bf16.

            sind rows are pre-rotated by 64 and sign-folded on the host."""
            cs = cos_sb[:, tpos : tpos + TT]
            sn = sin_sb[:, tpos : tpos + TT]
            tmp = tmppool.tile([128, TT], F32, tag="tmp", name="ropetmp")
            nc.vector.tensor_mul(tmp[0:64], pj[64:128], sn[64:128])
            nc.vector.tensor_mul(tmp[64:128], pj[0:64], sn[0:64])
            nc.vector.tensor_mul(dst, pj, cs)  # last psum read: frees the bank
            nc.vector.tensor_add(dst, dst, tmp)

        qT = [[None] * NT for _ in range(QH)]
        kT = [None] * NT
        v_sb = [None] * NT
        yT = [[None] * NT for _ in range(QH)]

        xcs = {}

        def emit_proj_kv(jt):
            tcol = jt * TT
            xbig = xpool.tile([128, KC, TT], BF16, tag="xc", name=f"xc_{jt}")
            xr = xT_ap[:, tcol : tcol + TT].rearrange("(kc p) t -> p kc t", p=128)
            nh = KC // 4
            for q in range(4):
                nc.sync.dma_start(
                    out=xbig[:, nh * q : nh * q + nh, :],
                    in_=xr[:, nh * q : nh * q + nh, :],
                )
            xc = xcs[jt] = [xbig[:, kc, :] for kc in range(KC)]
            if jt == 0:
                load_weights()
                load_late_consts()
            # k/v first: their weights (1MB) arrive long before wq (4MB)
            pk = ps.tile([128, TT], F32, tag="p", bufs=3, name=f"pk_{jt}")
            pv = ps.tile([128, TT], F32, tag="p", bufs=3, name=f"pv_{jt}")
            for kc in range(KC):
                st, sp = (kc == 0), (kc == KC - 1)
                nc.tensor.matmul(pk, wk_sb[kc], xc[kc], start=st, stop=sp)
                nc.tensor.matmul(pv, wv_sb[kc], xc[kc], start=st, stop=sp)
            kT[jt] = kpool.tile([128, TT], BF16, tag="kT", name=f"kT_{jt}")
            rope_evac(kT[jt], pk, tcol)
            vt_sb = tmppool.tile([128, TT], BF16, tag="vt", name=f"vt_{jt}")
            nc.scalar.copy(vt_sb, pv)  # frees the pv bank
            if jt == 0:
                load_wq(vt_sb[0:1, 0:1])
            vt_ps = ps.tile([128, 4, HD], BF16, tag="p", bufs=3, name=f"vtp_{jt}")
            for m in range(4):
                nc.tensor.transpose(
                    vt_ps[:, m, :], vt_sb[:, 128 * m : 128 * m + 128], id_sb
                )
            v_sb[jt] = vpool.tile([128, 4, HD], BF16, tag="v", name=f"v_{jt}")
            nc.vector.tensor_copy(v_sb[jt], vt_ps)

        def emit_proj_qpair(jt, hp):
            tcol = jt * TT
            xc = xcs[jt]
            if True:
                pq = [
                    ps.tile([128, TT], F32, tag="p", bufs=3, name=f"pq_{jt}_{hp}_{i}")
                    for i in range(2)
                ]
                for kc in range(KC):
                    st, sp = (kc == 0), (kc == KC - 1)
                    for i in range(2):
                        h = 2 * hp + i
                        nc.tensor.matmul(
                            pq[i],
                            wq_sb[kc][:, 128 * h : 128 * h + 128],
                            xc[kc],
                            start=st,
                            stop=sp,
                        )
                for i in range(2):
                    h = 2 * hp + i
                    qT[h][jt] = qkpool.tile(
                        [128, TT], BF16, tag="qT", name=f"qT_{h}_{jt}"
                    )
                    rope_evac(qT[h][jt], pq[i], tcol)

        def emit_proj(jt):
            emit_proj_kv(jt)
            emit_proj_qpair(jt, 0)
            emit_proj_qpair(jt, 1)

        def _finish_block(h, j, yp, zp):
            rcp = rcpool.tile([128, TT], F32, tag="rcp", bufs=3, name=f"rcp_{h}_{j}")
            nc.vector.reciprocal_approx_fast(out=rcp, in_=zp)
            yT[h][j] = ytpool.tile([128, TT], BF16, tag="yT", name=f"yT_{h}_{j}")
            nc.vector.tensor_mul(yT[h][j], yp, rcp)

        def emit_attn(h, j):
            """Single-head attention block (used while proj still owns psum)."""
            chs = _chunks(j)
            nch = len(chs)
            qTj = qT[h][j]
            yp = ps.tile([128, TT], F32, tag="acc", bufs=2, name=f"yp_{h}_{j}")
            zp = ps.tile([128, TT], F32, tag="acc", bufs=2, name=f"zp_{h}_{j}")
            pts = [None] * nch

            def emit_scores(i):
                cch, off = chs[i]
                sT = ps.tile([128, TT], F32, tag="s", bufs=3, name=f"sT_{h}_{j}_{i}")
                m = cch % 4
                nc.tensor.matmul(
                    sT[:, off:],
                    kT[cch // 4][:, 128 * m : 128 * m + 128],
                    qTj[:, off:],
                    start=True,
                    stop=True,
                )
                pT = ppool.tile([128, TT], BF16, tag="p", name=f"pT_{h}_{j}_{i}")
                nc.scalar.activation(
                    out=pT[:, off:],
                    in_=sT[:, off:],
                    func=mybir.ActivationFunctionType.Exp,
                    scale=SCALE,
                )
                if cch >= 4 * j:  # diagonal block: causal triangle
                    nc.gpsimd.tensor_mul(
                        pT[:, off : off + 128], pT[:, off : off + 128], tri_sb
                    )
                pts[i] = pT

            for i in range(min(LOOK, nch)):
                emit_scores(i)
            for i in range(nch):
                if i + LOOK < nch:
                    emit_scores(i + LOOK)
                cch, off = chs[i]
                pT = pts[i]
                st, sp = (i == 0), (i == nch - 1)
                nc.tensor.matmul(
                    yp[:, off:],
                    v_sb[cch // 4][:, cch % 4, :],
                    pT[:, off:],
                    start=st,
                    stop=sp,
                )
                nc.tensor.matmul(
                    zp[:, off:], ones_sb, pT[:, off:], start=st, stop=sp
                )
            _finish_block(h, j, yp, zp)

        def emit_attn2(h0, h1, j):
            """Two heads interleaved: PE streams head B while ACT exps head
            A, so the chunk cadence is PE-bound, not exp-latency-bound.
            Only for j >= 2, when projections no longer own the p/s tags."""
            chs = _chunks(j)
            nch = len(chs)
            acc = {
                h0: (
                    ps.tile([128, TT], F32, tag="acc", bufs=2, name=f"yp_{h0}_{j}"),
                    ps.tile([128, TT], F32, tag="acc", bufs=2, name=f"zp_{h0}_{j}"),
                ),
                h1: (
                    ps.tile([128, TT], F32, tag="s", bufs=3, name=f"yp_{h1}_{j}"),
                    ps.tile([128, TT], F32, tag="s", bufs=3, name=f"zp_{h1}_{j}"),
                ),
            }
            pts = {h0: [None] * nch, h1: [None] * nch}

            def emit_scores(h, i):
                cch, off = chs[i]
                sT = ps.tile([128, TT], F32, tag="p", bufs=3, name=f"sT_{h}_{j}_{i}")
                m = cch % 4
                nc.tensor.matmul(
                    sT[:, off:],
                    kT[cch // 4][:, 128 * m : 128 * m + 128],
                    qT[h][j][:, off:],
                    start=True,
                    stop=True,
                )
                pT = ppool.tile([128, TT], BF16, tag="p", name=f"pT_{h}_{j}_{i}")
                nc.scalar.activation(
                    out=pT[:, off:],
                    in_=sT[:, off:],
                    func=mybir.ActivationFunctionType.Exp,
                    scale=SCALE,
                )
                if cch >= 4 * j:
                    nc.gpsimd.tensor_mul(
                        pT[:, off : off + 128], pT[:, off : off + 128], tri_sb
                    )
                pts[h][i] = pT

            def emit_pv(h, i):
                cch, off = chs[i]
                yp, zp = acc[h]
                pT = pts[h][i]
                st, sp = (i == 0), (i == nch - 1)
                nc.tensor.matmul(
                    yp[:, off:],
                    v_sb[cch // 4][:, cch % 4, :],
                    pT[:, off:],
                    start=st,
                    stop=sp,
                )
                nc.tensor.matmul(
                    zp[:, off:], ones_sb, pT[:, off:], start=st, stop=sp
                )

            emit_scores(h0, 0)
            emit_scores(h1, 0)
            for i in range(nch):
                # both score matmuls first: stream A's exp gets five matmuls
                # of cover before pv(A, i) needs its result
                if i + 1 < nch:
                    emit_scores(h0, i + 1)
                    emit_scores(h1, i + 1)
                emit_pv(h0, i)
                emit_pv(h1, i)
            _finish_block(h0, j, *acc[h0])
            _finish_block(h1, j, *acc[h1])

        def emit_oproj(j):
            for ts_ in range(4 * j, 4 * j + 4):
                osb = outpool.tile([128, C], F16, tag="osb", name=f"osb_{ts_}")
                for n in range(C // TT):
                    op = ps.tile([128, TT], F32, tag="s", bufs=3, name=f"op_{ts_}_{n}")
                    for h in range(QH):
                        nc.tensor.matmul(
                            op,
                            yT[h][ts_ // 4][
                                :, 128 * (ts_ % 4) : 128 * (ts_ % 4) + 128
                            ],
                            wo_sb[:, h, TT * n : TT * n + TT],
                            start=(h == 0),
                            stop=(h == QH - 1),
                        )
                    if n % 2 == 1:
                        nc.scalar.copy(osb[:, TT * n : TT * n + TT], op)
                    else:
                        nc.vector.tensor_copy(osb[:, TT * n : TT * n + TT], op)
                    if n % 2 == 1:
                        eng = nc.gpsimd if ts_ % 2 == 0 else nc.sync
                        eng.dma_start(
                            out=out_ap[
                                128 * ts_ : 128 * ts_ + 128,
                                TT * (n - 1) : TT * (n + 1),
                            ],
                            in_=osb[:, TT * (n - 1) : TT * (n + 1)],
                        )

        # schedule: proj(j+2) fills the PE while attn(j) evac chains drain;
        # late attention runs two heads interleaved (exp no longer gates)
        emit_proj(0)
        emit_proj(1)
        emit_attn(0, 0)
        # wo (2MB) is first needed by o_proj(0); issuing it behind the first
        # attention block's tri-masks keeps it off the startup-critical path
        nc.gpsimd.dma_start(out=wo_sb, in_=wo.ap())
        for h in range(1, QH):
            emit_attn(h, 0)
        emit_proj(2)
        emit_oproj(0)
        emit_attn2(0, 1, 1)
        emit_attn2(2, 3, 1)
        emit_proj(3)
        emit_oproj(1)
        emit_attn2(0, 1, 2)
        emit_attn2(2, 3, 2)
        emit_oproj(2)
        emit_attn2(0, 1, 3)
        emit_attn2(2, 3, 3)
        emit_oproj(3)

    nc.finalize()
    return nc


_NC_CACHE = None
TRACE = False
LAST_RESULTS = None


def _get_nc():
    global _NC_CACHE
    if _NC_CACHE is None:
        _NC_CACHE = build_kernel()
    return _NC_CACHE


def kernel(x, Wq, Wk, Wv, Wo):
    bf16 = ml_dtypes.bfloat16
    x = np.asarray(x, dtype=np.float32)
    Wq = np.asarray(Wq, dtype=np.float32)
    Wk = np.asarray(Wk, dtype=np.float32)
    Wv = np.asarray(Wv, dtype=np.float32)
    Wo = np.asarray(Wo, dtype=np.float32)

    sin_, cos_ = _sin_cos_np()  # [T, 128]
    cosd = np.ascontiguousarray(cos_.T).astype(bf16)
    sinT = np.ascontiguousarray(sin_.T)
    # row-rotated by 64 and sign-folded: output rows 0:64 read input rows
    # 64:128 (value -sin), output rows 64:128 read input rows 0:64 (+sin)
    sind = np.empty_like(sinT)
    sind[64:128] = -sinT[0:64]
    sind[0:64] = sinT[64:128]
    sind = sind.astype(bf16)
    trid = np.triu(np.ones((128, 128), dtype=np.float32)).astype(bf16)
    onesd = np.ones((128, 128), dtype=bf16)
    identd = np.eye(128, dtype=np.float32).astype(bf16)

    xTb = [np.ascontiguousarray(x[b].T).astype(bf16) for b in range(B)]
    wq_g, wk_g, wv_g, wo_g = [], [], [], []
    for g in range(N_KV_HEAD):
        # [C, n] transposed slice, then tiled to [128, KC, n]
        wq_g.append(np.ascontiguousarray(
            Wq[QSH * g : QSH * (g + 1)].T.reshape(KC, 128, QSH).transpose(1, 0, 2)
        ).astype(bf16))
        wk_g.append(np.ascontiguousarray(
            Wk[HD * g : HD * (g + 1)].T.reshape(KC, 128, HD).transpose(1, 0, 2)
        ).astype(bf16))
        wv_g.append(np.ascontiguousarray(
            Wv[HD * g : HD * (g + 1)].T.reshape(KC, 128, HD).transpose(1, 0, 2)
        ).astype(bf16))
        # wo[p, h, f] = Wo[f, QSH*g + HD*h + p]
        woT = np.ascontiguousarray(Wo[:, QSH * g : QSH * (g + 1)].T)  # [512, C]
        wo_g.append(
            np.ascontiguousarray(
                woT.reshape(QH, HD, C).transpose(1, 0, 2)
            ).astype(bf16)
        )

    core_ids = list(range(N_CORES))
    in_maps = []
    for c in core_ids:
        b, g = c // N_KV_HEAD, c % N_KV_HEAD
        in_maps.append(
            {
                "xT": xTb[b],
                "wq": wq_g[g],
                "wk": wk_g[g],
                "wv": wv_g[g],
                "wo": wo_g[g],
                "cosd": cosd,
                "sind": sind,
                "trid": trid,
                "onesd": onesd,
                "identd": identd,
            }
        )
    global LAST_RESULTS
    res = run_bass_kernel_spmd(_get_nc(), in_maps, core_ids, trace=TRACE)
    LAST_RESULTS = res
    total = np.zeros((B, T, C), dtype=np.float32)
    for c in core_ids:
        total[c // N_KV_HEAD] += res.results[c]["out"].astype(np.float32)
    return total
